# revision 12
# baseline (speedup 1.0000x reference)
"""Bass/Tile TRN2 kernel for nn_BoxPairHead (bipartite GNN message passing).

Strategy (8 NeuronCores, u-dim row-sharded, 32 u rows per core):
  - Pair-MLP layer 1 is linear in the concat -> decompose:
      pair @ W1 = (eu @ W1a)[u] + (ev @ W1b)[v]
    so the giant [256*512, 1024] x [1024, 1024] matmul collapses into two
    small matmuls (AT, BT, kept transposed: [R, *]) plus a per-u broadcast
    add + relu done on ACT/DVE with per-partition bias.
  - Layer 2 stays transposed: C.T = W2.T @ h_u  (lhsT = W2 natural chunks).
  - Layer 3: per-u columns adjT[:, u] via lhsT = gT blocks, rhs = W3 chunk
    (N=1 matmuls are nearly free); sigmoid drains [128, 4] -> adjT.
  - v->u message: msg_vu = adj @ Mv (lhsT = adjT).
  - u->v message: partial = adj.T @ Mu per core; bf16 ReduceScatter over
    the 8 cores gives each core its 64-row v-slice of the summed
    [512, 1024]; LN + v-update are computed on the slice only; iter 1
    AllGathers the updated ev (bf16) for the next iteration's BT/Mv, and
    each core keeps its own slice for the v-update lhsT (no core-dependent
    indexing needed).
  - All matmuls bf16 (PSUM accumulation fp32); LayerNorm stats fp32.
"""

import sys

sys.path.insert(0, "/opt/trn_rl_repo")

import numpy as np
import ml_dtypes

import concourse.bass as bass
from concourse import bacc, mybir
from concourse.tile import TileContext
from concourse.bass_utils import run_bass_kernel_spmd
from concourse.masks import make_identity

F32 = mybir.dt.float32
BF16 = mybir.dt.bfloat16
AF = mybir.ActivationFunctionType
OP = mybir.AluOpType

N_CORES = 8
NU, NV, K, R, NUM_ITER = 256, 512, 512, 1024, 2
R2 = R // 2  # 512
S = NU // N_CORES  # 32 u rows per core
SV = NV // N_CORES  # 64 v rows per core
P = 128
KC = K // P  # 4 contraction chunks over the encoding dim
RC = R // P  # 8 chunks over R
R2C = R2 // P  # 4 chunks over R/2
VBN = NV // P  # 4 v blocks
NB = R // 512  # 2 free-dim blocks of 512 over R
EPS = 1e-5

BF16_NP = ml_dtypes.bfloat16


class _Balancer:
    """Round-robin DVE/ACT picker weighted by estimated op cost."""

    def __init__(self, nc):
        self.nc = nc
        self.busy = {"v": 0.0, "s": 0.0}

    def pick(self, v_cost, s_cost):
        if self.busy["v"] + v_cost <= self.busy["s"] + s_cost:
            self.busy["v"] += v_cost
            return self.nc.vector
        self.busy["s"] += s_cost
        return self.nc.scalar


def _build(b3_val: float):
    nc = bacc.Bacc("TRN2", target_bir_lowering=False, debug=False, num_devices=N_CORES)

    # ---- kernel I/O ----
    euT_in = nc.dram_tensor("euT_in", [K, S], BF16, kind="ExternalInput")
    evT_in = nc.dram_tensor("evT_in", [K, NV], BF16, kind="ExternalInput")
    w1a_in = nc.dram_tensor("w1a", [K, R], BF16, kind="ExternalInput")
    w1b_in = nc.dram_tensor("w1b", [K, R], BF16, kind="ExternalInput")
    w2_in = nc.dram_tensor("w2", [R, R2], BF16, kind="ExternalInput")
    w3_in = nc.dram_tensor("w3", [R2, 1], BF16, kind="ExternalInput")
    b1_in = nc.dram_tensor("b1c", [P, RC], F32, kind="ExternalInput")
    b2_in = nc.dram_tensor("b2c", [P, R2C], F32, kind="ExternalInput")
    vtou_in = nc.dram_tensor("vtouW", [K, R], BF16, kind="ExternalInput")
    utov_in = nc.dram_tensor("utovW", [K, R], BF16, kind="ExternalInput")
    uupd_in = nc.dram_tensor("uupdW", [K + R, K], BF16, kind="ExternalInput")
    vupd_in = nc.dram_tensor("vupdW", [K + R, K], BF16, kind="ExternalInput")

    eu_out = nc.dram_tensor("eu_out", [S, K], F32, kind="ExternalOutput")
    adj_out = nc.dram_tensor("adj_out", [S, NV], F32, kind="ExternalOutput")
    ev_out = nc.dram_tensor("ev_out", [SV, K], F32, kind="ExternalOutput")

    with TileContext(nc) as tc:
        with (
            tc.tile_pool(name="singles", bufs=1) as singles,
            tc.tile_pool(name="work", bufs=2) as work,
            tc.tile_pool(name="stats", bufs=4) as stats_pool,
            tc.tile_pool(name="psum", bufs=1, space="PSUM") as psum,
            tc.tile_pool(name="dram", bufs=2, space="DRAM") as dram,
        ):
            bal = _Balancer(nc)

            # ---- inputs first (pair loop needs them), then weights ----
            def load3(name, src, c, f, dtype=BF16):
                t = singles.tile([P, c, f], dtype, name=name)
                nc.sync.dma_start(out=t, in_=src[:, :].rearrange("(c p) f -> p c f", p=P))
                return t

            def load3_chunked(name, src, c, f, dtype=BF16):
                t = singles.tile([P, c, f], dtype, name=name)
                src3 = src[:, :].rearrange("(c p) f -> p c f", p=P)
                for ci in range(c):
                    nc.sync.dma_start(out=t[:, ci, :], in_=src3[:, ci, :])
                return t

            euT = work.tile([P, KC, S], BF16, tag="euT")
            nc.sync.dma_start(out=euT, in_=euT_in[:, :].rearrange("(c p) s -> p c s", p=P))
            w1a = load3_chunked("w1a_sb", w1a_in, KC, R)
            evT = work.tile([P, KC, NV], BF16, tag="evT")
            evT_src = evT_in[:, :].rearrange("(c p) v -> p c v", p=P)
            for ci in range(KC):
                nc.sync.dma_start(out=evT[:, ci, :], in_=evT_src[:, ci, :])
            w1b = load3_chunked("w1b_sb", w1b_in, KC, R)
            w2 = load3_chunked("w2_sb", w2_in, RC, R2)
            w3 = load3("w3_sb", w3_in, R2C, 1)
            b1 = singles.tile([P, RC], F32)
            nc.sync.dma_start(out=b1, in_=b1_in[:, :])
            b2 = singles.tile([P, R2C], F32)
            nc.sync.dma_start(out=b2, in_=b2_in[:, :])
            vtou = load3("vtou_sb", vtou_in, KC, R)
            utov = load3("utov_sb", utov_in, KC, R)
            uupd = load3("uupd_sb", uupd_in, KC + RC, K)
            vupd = load3("vupd_sb", vupd_in, KC + RC, K)

            iden = singles.tile([P, P], BF16)
            make_identity(nc, iden)
            eps_t = singles.tile([P, 1], F32)
            nc.vector.memset(eps_t, EPS)
            b3_t = singles.tile([P, 1], F32)
            nc.vector.memset(b3_t, b3_val)

            # ---- helpers ----
            def drain(dst, src, bias_col=None, relu=False, v_cost=0.6, s_cost=0.72):
                """PSUM/SBUF -> SBUF elementwise drain, optional +bias (per
                partition [P,1]) and relu, on whichever of DVE/ACT is freer."""
                eng = bal.pick(v_cost, s_cost)
                if eng is nc.vector:
                    if relu:
                        nc.vector.tensor_scalar(
                            out=dst, in0=src,
                            scalar1=bias_col if bias_col is not None else 0.0,
                            scalar2=0.0,
                            op0=OP.add, op1=OP.max,
                        )
                    elif bias_col is not None:
                        nc.vector.tensor_scalar_add(out=dst, in0=src, scalar1=bias_col)
                    else:
                        nc.vector.tensor_copy(out=dst, in_=src)
                else:
                    if relu:
                        nc.scalar.activation(
                            out=dst, in_=src, func=AF.Relu,
                            bias=bias_col if bias_col is not None else 0.0,
                        )
                    elif bias_col is not None:
                        nc.scalar.activation(out=dst, in_=src, func=AF.Identity, bias=bias_col)
                    else:
                        nc.scalar.copy(out=dst, in_=src)

            def transpose_to(dst, src):
                """src [p<=128, f<=128] SBUF bf16 -> dst [f, p] SBUF bf16."""
                pp = src.shape[0]
                ff = src.shape[-1]
                tp = psum.tile([P, P], src.dtype, tag="tp", bufs=1)
                nc.tensor.transpose(tp[:ff, :pp], src, iden[:pp, :pp])
                drain(dst, tp[:ff, :pp], v_cost=0.2, s_cost=0.45)

            def layer_norm(srcs, rows, outs):
                """LayerNorm over the concatenation of srcs (each [rows, <=512])
                along the free axis. outs: list of dst-lists, each aligned
                with srcs (multiple dst dtypes supported)."""
                nsub = len(srcs)
                st = stats_pool.tile([P, nsub, 6], F32, tag="st")
                for i, s in enumerate(srcs):
                    nc.vector.bn_stats(out=st[:rows, i, :], in_=s)
                mv = stats_pool.tile([P, 2], F32, tag="mv")
                nc.vector.bn_aggr(out=mv[:rows], in_=st[:rows])
                rstd = stats_pool.tile([P, 1], F32, tag="rstd")
                nc.scalar.activation(
                    out=rstd[:rows], in_=mv[:rows, 1:2], func=AF.Sqrt, bias=eps_t[:rows]
                )
                nc.vector.reciprocal(out=rstd[:rows], in_=rstd[:rows])
                for dsts in outs:
                    for i, s in enumerate(srcs):
                        nc.vector.tensor_scalar(
                            out=dsts[i], in0=s,
                            scalar1=mv[:rows, 0:1], scalar2=rstd[:rows],
                            op0=OP.subtract, op1=OP.mult,
                        )

            # ---- tiny warm-up collective: absorbs cross-core startup skew
            # and ncfw dispatch warmup while the PE runs the pair loop ----
            bar_sb = singles.tile([1, 16], BF16)
            nc.vector.memset(bar_sb, 0.0)
            bar_in = dram.tile([1, 16], BF16, tag="bar_in", bufs=1)
            nc.sync.dma_start(out=bar_in[:, :], in_=bar_sb)
            bar_out = dram.tile([N_CORES, 16], BF16, tag="bar_out", bufs=1, addr_space="Shared")
            nc.gpsimd.collective_compute(
                "AllGather",
                OP.bypass,
                replica_groups=[list(range(N_CORES))],
                ins=[bar_in[:, :]],
                outs=[bar_out[:, :]],
            )

            pid = nc.partition_id()
            voff = pid * SV

            # local v-slice of evT (columns [voff:voff+SV]) via dynamic DMA
            evTs = work.tile([P, KC, SV], BF16, tag="evTs")
            nc.sync.dma_start(
                out=evTs, in_=evT[:, :, bass.ds(voff, SV)]
            )

            # ================= iterations =================
            for it in range(NUM_ITER):
                last = it == NUM_ITER - 1

                # ---- AT[r, u] = (eu @ W1a).T ---- fp32 (used as bias operand)
                # traced first: on iter 2 it only depends on euT (ready before
                # the collective chain), so PE can run it during the AllGather.
                AT = work.tile([P, RC, S], F32, tag="AT", bufs=1)
                for rb in range(RC):
                    pa = psum.tile([P, S], F32, tag="tp", bufs=1)
                    for kc in range(KC):
                        nc.tensor.matmul(
                            pa, w1a[:, kc, bass.ts(rb, P)], euT[:, kc, :],
                            start=(kc == 0), stop=(kc == KC - 1),
                        )
                    drain(AT[:, rb, :], pa, v_cost=0.1, s_cost=0.35)

                # ---- BT[r, v] = (ev @ W1b).T + b1, pre-relu ---- [P, RC, NV]
                BT = work.tile([P, RC, NV], BF16, tag="BT", bufs=1)
                for rb in range(RC):
                    pb = psum.tile([P, 512], F32, tag="big", bufs=5)
                    for kc in range(KC):
                        nc.tensor.matmul(
                            pb, w1b[:, kc, bass.ts(rb, P)], evT[:, kc, :],
                            start=(kc == 0), stop=(kc == KC - 1),
                        )
                    drain(BT[:, rb, :], pb, bias_col=b1[:, rb : rb + 1])

                # ---- Mv[v, r] = relu(ev @ vtouW) ---- [P, VBN, R]
                Mv = work.tile([P, VBN, R], BF16, tag="Mv", bufs=1)
                for vb in range(VBN):
                    for nb in range(NB):
                        pm = psum.tile([P, 512], F32, tag="big", bufs=5)
                        for kc in range(KC):
                            nc.tensor.matmul(
                                pm, evT[:, kc, bass.ts(vb, P)], vtou[:, kc, bass.ts(nb, 512)],
                                start=(kc == 0), stop=(kc == KC - 1),
                            )
                        drain(Mv[:, vb, bass.ts(nb, 512)], pm, relu=True)

                # ---- pair-MLP u loop -> adjT ----
                adj_bf = work.tile([S, NV], BF16, tag="adj_bf", bufs=1)
                adjT = work.tile([P, VBN, S], BF16, tag="adjT", bufs=1)
                for u in range(S):
                    hT = work.tile([P, RC, NV], BF16, tag="hT", bufs=3)
                    for rc in range(RC):
                        drain(
                            hT[:, rc, :], BT[:, rc, :],
                            bias_col=AT[:, rc, u : u + 1], relu=True,
                            v_cost=0.33, s_cost=0.72,
                        )
                    gT = work.tile([P, R2C, NV], BF16, tag="gT")
                    for mb in range(R2C):
                        pc = psum.tile([P, 512], F32, tag="big", bufs=5)
                        for rc in range(RC):
                            nc.tensor.matmul(
                                pc, w2[:, rc, bass.ts(mb, P)], hT[:, rc, :],
                                start=(rc == 0), stop=(rc == RC - 1),
                            )
                        drain(gT[:, mb, :], pc, bias_col=b2[:, mb : mb + 1], relu=True)
                    pcol = psum.tile([P, VBN], F32, tag="row", bufs=2)
                    for vb in range(VBN):
                        for mb in range(R2C):
                            nc.tensor.matmul(
                                pcol[:, vb : vb + 1],
                                gT[:, mb, bass.ts(vb, P)], w3[:, mb, :],
                                start=(mb == 0), stop=(mb == R2C - 1),
                            )
                    nc.scalar.activation(
                        out=adjT[:, :, u], in_=pcol, func=AF.Sigmoid, bias=b3_t
                    )
                    bal.busy["s"] += 0.3

                # ---- adj natural (+ output) from adjT ----
                for vb in range(VBN):
                    transpose_to(adj_bf[:, bass.ts(vb, P)], adjT[:, vb, :])
                if last:
                    adj_f32 = work.tile([S, NV], F32, tag="adj_f32", bufs=1)
                    nc.vector.tensor_copy(out=adj_f32, in_=adj_bf)
                    nc.sync.dma_start(out=adj_out[:, :], in_=adj_f32)

                # ---- AllGather adj rows (factor 1 of the u->v reduction);
                # overlaps the msg_vu / u-update chain ----
                cc_adj_in = dram.tile([S, NV], BF16, tag="cc_adj_in")
                nc.sync.dma_start(out=cc_adj_in[:, :], in_=adj_bf)
                cc_adj = dram.tile([NU, NV], BF16, tag="cc_adj", addr_space="Shared")
                nc.gpsimd.collective_compute(
                    "AllGather",
                    OP.bypass,
                    replica_groups=[list(range(N_CORES))],
                    ins=[cc_adj_in[:, :]],
                    outs=[cc_adj[:, :]],
                )

                # ---- msg_vu = LN(adj @ Mv) ---- rows = S
                pmv = []
                for nb in range(NB):
                    pn = psum.tile([S, 512], F32, tag="row", bufs=2)
                    for vb in range(VBN):
                        nc.tensor.matmul(
                            pn, adjT[:, vb, :], Mv[:, vb, bass.ts(nb, 512)],
                            start=(vb == 0), stop=(vb == VBN - 1),
                        )
                    pmv.append(pn)
                msgvu = work.tile([S, R], BF16, tag="msgvu", bufs=1)
                layer_norm(pmv, S, [[msgvu[:, bass.ts(nb, 512)] for nb in range(NB)]])

                # ---- u update: eu = LN([eu, msg_vu] @ uupdW) ----
                msgvuT = work.tile([P, RC, S], BF16, tag="msgvuT", bufs=1)
                for rc in range(RC):
                    transpose_to(msgvuT[:, rc, :], msgvu[:, bass.ts(rc, P)])
                pe = psum.tile([S, K], F32, tag="row", bufs=2)
                for j in range(KC + RC):
                    lhsT = euT[:, j, :] if j < KC else msgvuT[:, j - KC, :]
                    nc.tensor.matmul(
                        pe, lhsT, uupd[:, j, :],
                        start=(j == 0), stop=(j == KC + RC - 1),
                    )
                eu_nat = work.tile([S, K], BF16, tag="eu_nat", bufs=1)
                ln_outs = [[eu_nat[:, :]]]
                if last:
                    eu_f32 = work.tile([S, K], F32, tag="eu_f32", bufs=1)
                    ln_outs.append([eu_f32[:, :]])
                layer_norm([pe], S, ln_outs)
                if last:
                    nc.sync.dma_start(out=eu_out[:, :], in_=eu_f32)

                euT2 = work.tile([P, KC, S], BF16, tag="euT")
                for kc in range(KC):
                    transpose_to(euT2[:, kc, :], eu_nat[:, bass.ts(kc, P)])
                euT = euT2

                # ---- Mu = relu(eu_new @ utovW) ---- [S, R]
                Mu = work.tile([S, R], BF16, tag="Mu", bufs=1)
                for nb in range(NB):
                    pu = psum.tile([S, 512], F32, tag="row", bufs=2)
                    for kc in range(KC):
                        nc.tensor.matmul(
                            pu, euT[:, kc, :], utov[:, kc, bass.ts(nb, 512)],
                            start=(kc == 0), stop=(kc == KC - 1),
                        )
                    drain(Mu[:, bass.ts(nb, 512)], pu, relu=True, v_cost=0.3, s_cost=0.72)

                # ---- AllGather Mu (factor 2), then recompute only the local
                # v-slice of msg_uv = adj_full[:, voff:voff+SV].T @ Mu_full ----
                cc_mu_in = dram.tile([S, R], BF16, tag="cc_mu_in")
                nc.sync.dma_start(out=cc_mu_in[:, :], in_=Mu)
                cc_mu = dram.tile([NU, R], BF16, tag="cc_mu", addr_space="Shared")
                nc.gpsimd.collective_compute(
                    "AllGather",
                    OP.bypass,
                    replica_groups=[list(range(N_CORES))],
                    ins=[cc_mu_in[:, :]],
                    outs=[cc_mu[:, :]],
                )
                muF = work.tile([P, NU // P, R], BF16, tag="muF", bufs=1)
                nc.sync.dma_start(
                    out=muF, in_=cc_mu[:, :].rearrange("(c p) r -> p c r", p=P)
                )
                adjS = work.tile([P, NU // P, SV], BF16, tag="adjS", bufs=1)
                nc.sync.dma_start(
                    out=adjS,
                    in_=cc_adj[:, :].rearrange("(c p) v -> p c v", p=P)[
                        :, :, bass.ds(voff, SV)
                    ],
                )
                pmuv = []
                for nb in range(NB):
                    pw = psum.tile([SV, 512], F32, tag="row", bufs=2)
                    for c in range(NU // P):
                        nc.tensor.matmul(
                            pw, adjS[:, c, :], muF[:, c, bass.ts(nb, 512)],
                            start=(c == 0), stop=(c == NU // P - 1),
                        )
                    pmuv.append(pw)

                # ---- LN of the local v-slice of msg_uv ----
                mslice_n = work.tile([SV, R], BF16, tag="mslice_n", bufs=1)
                layer_norm(
                    pmuv,
                    SV,
                    [[mslice_n[:, bass.ts(nb, 512)] for nb in range(NB)]],
                )
                msguvT = work.tile([P, RC, SV], BF16, tag="msguvT", bufs=1)
                for rc in range(RC):
                    transpose_to(msguvT[:, rc, :], mslice_n[:, bass.ts(rc, P)])

                # ---- v update (local slice): ev_s = LN([ev_s, msg_uv_s] @ vupdW) ----
                pv = psum.tile([SV, K], F32, tag="row", bufs=2)
                for j in range(KC + RC):
                    lhsT = evTs[:, j, :] if j < KC else msguvT[:, j - KC, :]
                    nc.tensor.matmul(
                        pv, lhsT, vupd[:, j, :],
                        start=(j == 0), stop=(j == KC + RC - 1),
                    )
                if last:
                    evs_f32 = work.tile([SV, K], F32, tag="evs_f32", bufs=1)
                    layer_norm([pv], SV, [[evs_f32[:, :]]])
                    nc.sync.dma_start(out=ev_out[:, :], in_=evs_f32)
                else:
                    evs_nat = work.tile([SV, K], BF16, tag="evs_nat", bufs=1)
                    layer_norm([pv], SV, [[evs_nat[:, :]]])
                    # AllGather the updated ev for the next iteration's BT/Mv
                    cc_agin = dram.tile([SV, K], BF16, tag="cc_agin")
                    nc.sync.dma_start(out=cc_agin[:, :], in_=evs_nat)
                    cc_ag = dram.tile([NV, K], BF16, tag="cc_ag", addr_space="Shared")
                    nc.gpsimd.collective_compute(
                        "AllGather",
                        OP.bypass,
                        replica_groups=[list(range(N_CORES))],
                        ins=[cc_agin[:, :]],
                        outs=[cc_ag[:, :]],
                    )
                    ev_nat = work.tile([P, VBN, K], BF16, tag="ev_nat", bufs=1)
                    nc.sync.dma_start(
                        out=ev_nat, in_=cc_ag[:, :].rearrange("(vb p) k -> p vb k", p=P)
                    )
                    evT2 = work.tile([P, KC, NV], BF16, tag="evT")
                    for kc in range(KC):
                        for vb in range(VBN):
                            transpose_to(
                                evT2[:, kc, bass.ts(vb, P)],
                                ev_nat[:, vb, bass.ts(kc, P)],
                            )
                    evT = evT2
                    # next iteration's local evT slice (lhsT of the v update)
                    evTs2 = work.tile([P, KC, SV], BF16, tag="evTs")
                    nc.sync.dma_start(
                        out=evTs2, in_=evT[:, :, bass.ds(voff, SV)]
                    )
                    evTs = evTs2

    nc.compile()
    return nc


_CACHE = {}


def _get_nc(b3_val: float):
    key = float(b3_val)
    if key not in _CACHE:
        _CACHE[key] = _build(key)
    return _CACHE[key]


def kernel(
    encodings_u, encodings_v, adjW1, adjb1, adjW2, adjb2, adjW3, adjb3,
    utovW, utovb, vtouW, vtoub, utov_g, utov_b, vtou_g, vtou_b,
    uupdW, uupd_g, uupd_b, vupdW, vupd_g, vupd_b,
):
    f32 = np.float32

    def np32(x):
        return np.asarray(x, dtype=f32)

    # The device kernel folds the trivial (zero/one) affine params away;
    # assert they really are trivial for this problem instance.
    for name, arr, val in [
        ("utovb", utovb, 0.0), ("vtoub", vtoub, 0.0),
        ("utov_g", utov_g, 1.0), ("utov_b", utov_b, 0.0),
        ("vtou_g", vtou_g, 1.0), ("vtou_b", vtou_b, 0.0),
        ("uupd_g", uupd_g, 1.0), ("uupd_b", uupd_b, 0.0),
        ("vupd_g", vupd_g, 1.0), ("vupd_b", vupd_b, 0.0),
    ]:
        if not np.allclose(np32(arr), val, atol=1e-30):
            raise NotImplementedError(f"nontrivial {name} not supported")

    eu = np32(encodings_u)
    ev = np32(encodings_v)
    W1 = np32(adjW1)
    b1 = np32(adjb1)
    b3_val = float(np32(adjb3).reshape(-1)[0])

    def bf(x):
        return np.ascontiguousarray(x).astype(BF16_NP)

    evT = bf(ev.T)
    shared = {
        "evT_in": evT,
        "w1a": bf(W1[:K]),
        "w1b": bf(W1[K:]),
        "w2": bf(np32(adjW2)),
        "w3": bf(np32(adjW3)),
        "b1c": np.ascontiguousarray(b1.reshape(RC, P).T).astype(f32),
        "b2c": np.ascontiguousarray(np32(adjb2).reshape(R2C, P).T).astype(f32),
        "vtouW": bf(np32(vtouW)),
        "utovW": bf(np32(utovW)),
        "uupdW": bf(np32(uupdW)),
        "vupdW": bf(np32(vupdW)),
    }
    in_maps = []
    for c in range(N_CORES):
        m = dict(shared)
        m["euT_in"] = bf(eu[c * S : (c + 1) * S].T)
        in_maps.append(m)

    nc = _get_nc(b3_val)
    res = run_bass_kernel_spmd(nc, in_maps, core_ids=list(range(N_CORES)))
    eu_full = np.concatenate([res.results[c]["eu_out"] for c in range(N_CORES)], axis=0)
    adj_full = np.concatenate([res.results[c]["adj_out"] for c in range(N_CORES)], axis=0)
    ev_full = np.concatenate([res.results[c]["ev_out"] for c in range(N_CORES)], axis=0)
    return eu_full.astype(f32), ev_full.astype(f32), adj_full.astype(f32)


# revision 13
# speedup vs baseline: 1.0279x; 1.0279x over previous
"""Bass/Tile TRN2 kernel for nn_BoxPairHead (bipartite GNN message passing).

Strategy (8 NeuronCores, u-dim row-sharded, 32 u rows per core):
  - Pair-MLP layer 1 is linear in the concat -> decompose:
      pair @ W1 = (eu @ W1a)[u] + (ev @ W1b)[v]
    so the giant [256*512, 1024] x [1024, 1024] matmul collapses into two
    small matmuls (AT, BT, kept transposed: [R, *]) plus a per-u broadcast
    add + relu done on ACT/DVE with per-partition bias.
  - Layer 2 stays transposed: C.T = W2.T @ h_u  (lhsT = W2 natural chunks).
  - Layer 3: per-u columns adjT[:, u] via lhsT = gT blocks, rhs = W3 chunk
    (N=1 matmuls are nearly free); sigmoid drains [128, 4] -> adjT.
  - v->u message: msg_vu = adj @ Mv (lhsT = adjT).
  - u->v message: partial = adj.T @ Mu per core; bf16 ReduceScatter over
    the 8 cores gives each core its 64-row v-slice of the summed
    [512, 1024]; LN + v-update are computed on the slice only; iter 1
    AllGathers the updated ev (bf16) for the next iteration's BT/Mv, and
    each core keeps its own slice for the v-update lhsT (no core-dependent
    indexing needed).
  - All matmuls bf16 (PSUM accumulation fp32); LayerNorm stats fp32.
"""

import sys

sys.path.insert(0, "/opt/trn_rl_repo")

import numpy as np
import ml_dtypes

import concourse.bass as bass
from concourse import bacc, mybir
from concourse.tile import TileContext
from concourse.bass_utils import run_bass_kernel_spmd
from concourse.masks import make_identity

F32 = mybir.dt.float32
BF16 = mybir.dt.bfloat16
AF = mybir.ActivationFunctionType
OP = mybir.AluOpType

N_CORES = 8
NU, NV, K, R, NUM_ITER = 256, 512, 512, 1024, 2
R2 = R // 2  # 512
S = NU // N_CORES  # 32 u rows per core
SV = NV // N_CORES  # 64 v rows per core
P = 128
KC = K // P  # 4 contraction chunks over the encoding dim
RC = R // P  # 8 chunks over R
R2C = R2 // P  # 4 chunks over R/2
VBN = NV // P  # 4 v blocks
NB = R // 512  # 2 free-dim blocks of 512 over R
EPS = 1e-5

BF16_NP = ml_dtypes.bfloat16


class _Balancer:
    """Round-robin DVE/ACT picker weighted by estimated op cost."""

    def __init__(self, nc):
        self.nc = nc
        self.busy = {"v": 0.0, "s": 0.0}

    def pick(self, v_cost, s_cost):
        if self.busy["v"] + v_cost <= self.busy["s"] + s_cost:
            self.busy["v"] += v_cost
            return self.nc.vector
        self.busy["s"] += s_cost
        return self.nc.scalar


def _build(b3_val: float):
    nc = bacc.Bacc("TRN2", target_bir_lowering=False, debug=False, num_devices=N_CORES)

    # ---- kernel I/O ----
    euT_in = nc.dram_tensor("euT_in", [K, S], BF16, kind="ExternalInput")
    evT_in = nc.dram_tensor("evT_in", [K, NV], BF16, kind="ExternalInput")
    w1a_in = nc.dram_tensor("w1a", [K, R], BF16, kind="ExternalInput")
    w1b_in = nc.dram_tensor("w1b", [K, R], BF16, kind="ExternalInput")
    w2_in = nc.dram_tensor("w2", [R, R2], BF16, kind="ExternalInput")
    w3_in = nc.dram_tensor("w3", [R2, 1], BF16, kind="ExternalInput")
    b1_in = nc.dram_tensor("b1c", [P, RC], F32, kind="ExternalInput")
    b2_in = nc.dram_tensor("b2c", [P, R2C], F32, kind="ExternalInput")
    vtou_in = nc.dram_tensor("vtouW", [K, R], BF16, kind="ExternalInput")
    utov_in = nc.dram_tensor("utovW", [K, R], BF16, kind="ExternalInput")
    uupd_in = nc.dram_tensor("uupdW", [K + R, K], BF16, kind="ExternalInput")
    vupd_in = nc.dram_tensor("vupdW", [K + R, K], BF16, kind="ExternalInput")

    eu_out = nc.dram_tensor("eu_out", [S, K], F32, kind="ExternalOutput")
    adj_out = nc.dram_tensor("adj_out", [S, NV], F32, kind="ExternalOutput")
    ev_out = nc.dram_tensor("ev_out", [SV, K], F32, kind="ExternalOutput")

    with TileContext(nc) as tc:
        with (
            tc.tile_pool(name="singles", bufs=1) as singles,
            tc.tile_pool(name="work", bufs=2) as work,
            tc.tile_pool(name="stats", bufs=4) as stats_pool,
            tc.tile_pool(name="psum", bufs=1, space="PSUM") as psum,
            tc.tile_pool(name="dram", bufs=2, space="DRAM") as dram,
        ):
            bal = _Balancer(nc)

            # ---- inputs first (pair loop needs them), then weights ----
            def load3(name, src, c, f, dtype=BF16):
                t = singles.tile([P, c, f], dtype, name=name)
                nc.sync.dma_start(out=t, in_=src[:, :].rearrange("(c p) f -> p c f", p=P))
                return t

            euT = work.tile([P, KC, S], BF16, tag="euT")
            nc.sync.dma_start(out=euT, in_=euT_in[:, :].rearrange("(c p) s -> p c s", p=P))
            w1a = load3("w1a_sb", w1a_in, KC, R)
            evT = work.tile([P, KC, NV], BF16, tag="evT")
            nc.sync.dma_start(out=evT, in_=evT_in[:, :].rearrange("(c p) v -> p c v", p=P))
            w1b = load3("w1b_sb", w1b_in, KC, R)
            w2 = load3("w2_sb", w2_in, RC, R2)
            w3 = load3("w3_sb", w3_in, R2C, 1)
            b1 = singles.tile([P, RC], F32)
            nc.sync.dma_start(out=b1, in_=b1_in[:, :])
            b2 = singles.tile([P, R2C], F32)
            nc.sync.dma_start(out=b2, in_=b2_in[:, :])
            vtou = load3("vtou_sb", vtou_in, KC, R)
            utov = load3("utov_sb", utov_in, KC, R)
            uupd = load3("uupd_sb", uupd_in, KC + RC, K)
            vupd = load3("vupd_sb", vupd_in, KC + RC, K)

            iden = singles.tile([P, P], BF16)
            make_identity(nc, iden)
            eps_t = singles.tile([P, 1], F32)
            nc.vector.memset(eps_t, EPS)
            b3_t = singles.tile([P, 1], F32)
            nc.vector.memset(b3_t, b3_val)

            # ---- helpers ----
            def drain(dst, src, bias_col=None, relu=False, v_cost=0.6, s_cost=0.72):
                """PSUM/SBUF -> SBUF elementwise drain, optional +bias (per
                partition [P,1]) and relu, on whichever of DVE/ACT is freer."""
                eng = bal.pick(v_cost, s_cost)
                if eng is nc.vector:
                    if relu:
                        nc.vector.tensor_scalar(
                            out=dst, in0=src,
                            scalar1=bias_col if bias_col is not None else 0.0,
                            scalar2=0.0,
                            op0=OP.add, op1=OP.max,
                        )
                    elif bias_col is not None:
                        nc.vector.tensor_scalar_add(out=dst, in0=src, scalar1=bias_col)
                    else:
                        nc.vector.tensor_copy(out=dst, in_=src)
                else:
                    if relu:
                        nc.scalar.activation(
                            out=dst, in_=src, func=AF.Relu,
                            bias=bias_col if bias_col is not None else 0.0,
                        )
                    elif bias_col is not None:
                        nc.scalar.activation(out=dst, in_=src, func=AF.Identity, bias=bias_col)
                    else:
                        nc.scalar.copy(out=dst, in_=src)

            def transpose_to(dst, src):
                """src [p<=128, f<=128] SBUF bf16 -> dst [f, p] SBUF bf16."""
                pp = src.shape[0]
                ff = src.shape[-1]
                tp = psum.tile([P, P], src.dtype, tag="tp", bufs=1)
                nc.tensor.transpose(tp[:ff, :pp], src, iden[:pp, :pp])
                drain(dst, tp[:ff, :pp], v_cost=0.2, s_cost=0.45)

            def layer_norm(srcs, rows, outs):
                """LayerNorm over the concatenation of srcs (each [rows, <=512])
                along the free axis. outs: list of dst-lists, each aligned
                with srcs (multiple dst dtypes supported)."""
                nsub = len(srcs)
                st = stats_pool.tile([P, nsub, 6], F32, tag="st")
                for i, s in enumerate(srcs):
                    nc.vector.bn_stats(out=st[:rows, i, :], in_=s)
                mv = stats_pool.tile([P, 2], F32, tag="mv")
                nc.vector.bn_aggr(out=mv[:rows], in_=st[:rows])
                rstd = stats_pool.tile([P, 1], F32, tag="rstd")
                nc.scalar.activation(
                    out=rstd[:rows], in_=mv[:rows, 1:2], func=AF.Sqrt, bias=eps_t[:rows]
                )
                nc.vector.reciprocal(out=rstd[:rows], in_=rstd[:rows])
                for dsts in outs:
                    for i, s in enumerate(srcs):
                        nc.vector.tensor_scalar(
                            out=dsts[i], in0=s,
                            scalar1=mv[:rows, 0:1], scalar2=rstd[:rows],
                            op0=OP.subtract, op1=OP.mult,
                        )

            # ---- tiny warm-up collective: absorbs cross-core startup skew
            # and ncfw dispatch warmup while the PE runs the pair loop ----
            bar_sb = singles.tile([1, 16], BF16)
            nc.vector.memset(bar_sb, 0.0)
            bar_in = dram.tile([1, 16], BF16, tag="bar_in", bufs=1)
            nc.sync.dma_start(out=bar_in[:, :], in_=bar_sb)
            bar_out = dram.tile([N_CORES, 16], BF16, tag="bar_out", bufs=1, addr_space="Shared")
            nc.gpsimd.collective_compute(
                "AllGather",
                OP.bypass,
                replica_groups=[list(range(N_CORES))],
                ins=[bar_in[:, :]],
                outs=[bar_out[:, :]],
            )

            pid = nc.partition_id()
            voff = pid * SV

            # local v-slice of evT (columns [voff:voff+SV]) via dynamic DMA
            evTs = work.tile([P, KC, SV], BF16, tag="evTs")
            nc.sync.dma_start(
                out=evTs, in_=evT[:, :, bass.ds(voff, SV)]
            )

            # ================= iterations =================
            for it in range(NUM_ITER):
                last = it == NUM_ITER - 1

                # ---- AT[r, u] = (eu @ W1a).T ---- fp32 (used as bias operand)
                # traced first: on iter 2 it only depends on euT (ready before
                # the collective chain), so PE can run it during the AllGather.
                AT = work.tile([P, RC, S], F32, tag="AT", bufs=1)
                for rb in range(RC):
                    pa = psum.tile([P, S], F32, tag="tp", bufs=1)
                    for kc in range(KC):
                        nc.tensor.matmul(
                            pa, w1a[:, kc, bass.ts(rb, P)], euT[:, kc, :],
                            start=(kc == 0), stop=(kc == KC - 1),
                        )
                    drain(AT[:, rb, :], pa, v_cost=0.1, s_cost=0.35)

                # ---- BT[r, v] = (ev @ W1b).T + b1, pre-relu ---- [P, RC, NV]
                BT = work.tile([P, RC, NV], BF16, tag="BT", bufs=1)
                for rb in range(RC):
                    pb = psum.tile([P, 512], F32, tag="big", bufs=5)
                    for kc in range(KC):
                        nc.tensor.matmul(
                            pb, w1b[:, kc, bass.ts(rb, P)], evT[:, kc, :],
                            start=(kc == 0), stop=(kc == KC - 1),
                        )
                    drain(BT[:, rb, :], pb, bias_col=b1[:, rb : rb + 1])

                # ---- Mv[v, r] = relu(ev @ vtouW) ---- [P, VBN, R]
                Mv = work.tile([P, VBN, R], BF16, tag="Mv", bufs=1)
                for vb in range(VBN):
                    for nb in range(NB):
                        pm = psum.tile([P, 512], F32, tag="big", bufs=5)
                        for kc in range(KC):
                            nc.tensor.matmul(
                                pm, evT[:, kc, bass.ts(vb, P)], vtou[:, kc, bass.ts(nb, 512)],
                                start=(kc == 0), stop=(kc == KC - 1),
                            )
                        drain(Mv[:, vb, bass.ts(nb, 512)], pm, relu=True)

                # ---- pair-MLP u loop -> adjT ----
                adj_bf = work.tile([S, NV], BF16, tag="adj_bf", bufs=1)
                adjT = work.tile([P, VBN, S], BF16, tag="adjT", bufs=1)
                for u in range(S):
                    hT = work.tile([P, RC, NV], BF16, tag="hT", bufs=3)
                    for rc in range(RC):
                        drain(
                            hT[:, rc, :], BT[:, rc, :],
                            bias_col=AT[:, rc, u : u + 1], relu=True,
                            v_cost=0.33, s_cost=0.72,
                        )
                    gT = work.tile([P, R2C, NV], BF16, tag="gT")
                    for mb in range(R2C):
                        pc = psum.tile([P, 512], F32, tag="big", bufs=5)
                        for rc in range(RC):
                            nc.tensor.matmul(
                                pc, w2[:, rc, bass.ts(mb, P)], hT[:, rc, :],
                                start=(rc == 0), stop=(rc == RC - 1),
                            )
                        drain(gT[:, mb, :], pc, bias_col=b2[:, mb : mb + 1], relu=True)
                    pcol = psum.tile([P, VBN], F32, tag="row", bufs=2)
                    for vb in range(VBN):
                        for mb in range(R2C):
                            nc.tensor.matmul(
                                pcol[:, vb : vb + 1],
                                gT[:, mb, bass.ts(vb, P)], w3[:, mb, :],
                                start=(mb == 0), stop=(mb == R2C - 1),
                            )
                    nc.scalar.activation(
                        out=adjT[:, :, u], in_=pcol, func=AF.Sigmoid, bias=b3_t
                    )
                    bal.busy["s"] += 0.3

                # ---- adj natural (+ output) from adjT ----
                for vb in range(VBN):
                    transpose_to(adj_bf[:, bass.ts(vb, P)], adjT[:, vb, :])
                if last:
                    adj_f32 = work.tile([S, NV], F32, tag="adj_f32", bufs=1)
                    nc.vector.tensor_copy(out=adj_f32, in_=adj_bf)
                    nc.sync.dma_start(out=adj_out[:, :], in_=adj_f32)

                # ---- AllGather adj rows (factor 1 of the u->v reduction);
                # overlaps the msg_vu / u-update chain ----
                cc_adj_in = dram.tile([S, NV], BF16, tag="cc_adj_in")
                nc.sync.dma_start(out=cc_adj_in[:, :], in_=adj_bf)
                cc_adj = dram.tile([NU, NV], BF16, tag="cc_adj", addr_space="Shared")
                nc.gpsimd.collective_compute(
                    "AllGather",
                    OP.bypass,
                    replica_groups=[list(range(N_CORES))],
                    ins=[cc_adj_in[:, :]],
                    outs=[cc_adj[:, :]],
                )

                # ---- msg_vu = LN(adj @ Mv) ---- rows = S
                pmv = []
                for nb in range(NB):
                    pn = psum.tile([S, 512], F32, tag="row", bufs=2)
                    for vb in range(VBN):
                        nc.tensor.matmul(
                            pn, adjT[:, vb, :], Mv[:, vb, bass.ts(nb, 512)],
                            start=(vb == 0), stop=(vb == VBN - 1),
                        )
                    pmv.append(pn)
                msgvu = work.tile([S, R], BF16, tag="msgvu", bufs=1)
                layer_norm(pmv, S, [[msgvu[:, bass.ts(nb, 512)] for nb in range(NB)]])

                # ---- u update: eu = LN([eu, msg_vu] @ uupdW) ----
                msgvuT = work.tile([P, RC, S], BF16, tag="msgvuT", bufs=1)
                for rc in range(RC):
                    transpose_to(msgvuT[:, rc, :], msgvu[:, bass.ts(rc, P)])
                pe = psum.tile([S, K], F32, tag="row", bufs=2)
                for j in range(KC + RC):
                    lhsT = euT[:, j, :] if j < KC else msgvuT[:, j - KC, :]
                    nc.tensor.matmul(
                        pe, lhsT, uupd[:, j, :],
                        start=(j == 0), stop=(j == KC + RC - 1),
                    )
                eu_nat = work.tile([S, K], BF16, tag="eu_nat", bufs=1)
                ln_outs = [[eu_nat[:, :]]]
                if last:
                    eu_f32 = work.tile([S, K], F32, tag="eu_f32", bufs=1)
                    ln_outs.append([eu_f32[:, :]])
                layer_norm([pe], S, ln_outs)
                if last:
                    nc.sync.dma_start(out=eu_out[:, :], in_=eu_f32)

                euT2 = work.tile([P, KC, S], BF16, tag="euT")
                for kc in range(KC):
                    transpose_to(euT2[:, kc, :], eu_nat[:, bass.ts(kc, P)])
                euT = euT2

                # ---- Mu = relu(eu_new @ utovW) ---- [S, R]
                Mu = work.tile([S, R], BF16, tag="Mu", bufs=1)
                for nb in range(NB):
                    pu = psum.tile([S, 512], F32, tag="row", bufs=2)
                    for kc in range(KC):
                        nc.tensor.matmul(
                            pu, euT[:, kc, :], utov[:, kc, bass.ts(nb, 512)],
                            start=(kc == 0), stop=(kc == KC - 1),
                        )
                    drain(Mu[:, bass.ts(nb, 512)], pu, relu=True, v_cost=0.3, s_cost=0.72)

                # ---- AllGather Mu (factor 2), then recompute only the local
                # v-slice of msg_uv = adj_full[:, voff:voff+SV].T @ Mu_full ----
                cc_mu_in = dram.tile([S, R], BF16, tag="cc_mu_in")
                nc.sync.dma_start(out=cc_mu_in[:, :], in_=Mu)
                cc_mu = dram.tile([NU, R], BF16, tag="cc_mu", addr_space="Shared")
                nc.gpsimd.collective_compute(
                    "AllGather",
                    OP.bypass,
                    replica_groups=[list(range(N_CORES))],
                    ins=[cc_mu_in[:, :]],
                    outs=[cc_mu[:, :]],
                )
                muF = work.tile([P, NU // P, R], BF16, tag="muF", bufs=1)
                nc.sync.dma_start(
                    out=muF, in_=cc_mu[:, :].rearrange("(c p) r -> p c r", p=P)
                )
                adjS = work.tile([P, NU // P, SV], BF16, tag="adjS", bufs=1)
                nc.sync.dma_start(
                    out=adjS,
                    in_=cc_adj[:, :].rearrange("(c p) v -> p c v", p=P)[
                        :, :, bass.ds(voff, SV)
                    ],
                )
                pmuv = []
                for nb in range(NB):
                    pw = psum.tile([SV, 512], F32, tag="row", bufs=2)
                    for c in range(NU // P):
                        nc.tensor.matmul(
                            pw, adjS[:, c, :], muF[:, c, bass.ts(nb, 512)],
                            start=(c == 0), stop=(c == NU // P - 1),
                        )
                    pmuv.append(pw)

                # ---- LN of the local v-slice of msg_uv ----
                mslice_n = work.tile([SV, R], BF16, tag="mslice_n", bufs=1)
                layer_norm(
                    pmuv,
                    SV,
                    [[mslice_n[:, bass.ts(nb, 512)] for nb in range(NB)]],
                )
                msguvT = work.tile([P, RC, SV], BF16, tag="msguvT", bufs=1)
                for rc in range(RC):
                    transpose_to(msguvT[:, rc, :], mslice_n[:, bass.ts(rc, P)])

                # ---- v update (local slice): ev_s = LN([ev_s, msg_uv_s] @ vupdW) ----
                pv = psum.tile([SV, K], F32, tag="row", bufs=2)
                for j in range(KC + RC):
                    lhsT = evTs[:, j, :] if j < KC else msguvT[:, j - KC, :]
                    nc.tensor.matmul(
                        pv, lhsT, vupd[:, j, :],
                        start=(j == 0), stop=(j == KC + RC - 1),
                    )
                if last:
                    evs_f32 = work.tile([SV, K], F32, tag="evs_f32", bufs=1)
                    layer_norm([pv], SV, [[evs_f32[:, :]]])
                    nc.sync.dma_start(out=ev_out[:, :], in_=evs_f32)
                else:
                    evs_nat = work.tile([SV, K], BF16, tag="evs_nat", bufs=1)
                    layer_norm([pv], SV, [[evs_nat[:, :]]])
                    # AllGather the updated ev for the next iteration's BT/Mv
                    cc_agin = dram.tile([SV, K], BF16, tag="cc_agin")
                    nc.sync.dma_start(out=cc_agin[:, :], in_=evs_nat)
                    cc_ag = dram.tile([NV, K], BF16, tag="cc_ag", addr_space="Shared")
                    nc.gpsimd.collective_compute(
                        "AllGather",
                        OP.bypass,
                        replica_groups=[list(range(N_CORES))],
                        ins=[cc_agin[:, :]],
                        outs=[cc_ag[:, :]],
                    )
                    ev_nat = work.tile([P, VBN, K], BF16, tag="ev_nat", bufs=1)
                    nc.sync.dma_start(
                        out=ev_nat, in_=cc_ag[:, :].rearrange("(vb p) k -> p vb k", p=P)
                    )
                    evT2 = work.tile([P, KC, NV], BF16, tag="evT")
                    for kc in range(KC):
                        for vb in range(VBN):
                            transpose_to(
                                evT2[:, kc, bass.ts(vb, P)],
                                ev_nat[:, vb, bass.ts(kc, P)],
                            )
                    evT = evT2
                    # next iteration's local evT slice (lhsT of the v update)
                    evTs2 = work.tile([P, KC, SV], BF16, tag="evTs")
                    nc.sync.dma_start(
                        out=evTs2, in_=evT[:, :, bass.ds(voff, SV)]
                    )
                    evTs = evTs2

    nc.compile()
    return nc


_CACHE = {}


def _get_nc(b3_val: float):
    key = float(b3_val)
    if key not in _CACHE:
        _CACHE[key] = _build(key)
    return _CACHE[key]


def kernel(
    encodings_u, encodings_v, adjW1, adjb1, adjW2, adjb2, adjW3, adjb3,
    utovW, utovb, vtouW, vtoub, utov_g, utov_b, vtou_g, vtou_b,
    uupdW, uupd_g, uupd_b, vupdW, vupd_g, vupd_b,
):
    f32 = np.float32

    def np32(x):
        return np.asarray(x, dtype=f32)

    # The device kernel folds the trivial (zero/one) affine params away;
    # assert they really are trivial for this problem instance.
    for name, arr, val in [
        ("utovb", utovb, 0.0), ("vtoub", vtoub, 0.0),
        ("utov_g", utov_g, 1.0), ("utov_b", utov_b, 0.0),
        ("vtou_g", vtou_g, 1.0), ("vtou_b", vtou_b, 0.0),
        ("uupd_g", uupd_g, 1.0), ("uupd_b", uupd_b, 0.0),
        ("vupd_g", vupd_g, 1.0), ("vupd_b", vupd_b, 0.0),
    ]:
        if not np.allclose(np32(arr), val, atol=1e-30):
            raise NotImplementedError(f"nontrivial {name} not supported")

    eu = np32(encodings_u)
    ev = np32(encodings_v)
    W1 = np32(adjW1)
    b1 = np32(adjb1)
    b3_val = float(np32(adjb3).reshape(-1)[0])

    def bf(x):
        return np.ascontiguousarray(x).astype(BF16_NP)

    evT = bf(ev.T)
    shared = {
        "evT_in": evT,
        "w1a": bf(W1[:K]),
        "w1b": bf(W1[K:]),
        "w2": bf(np32(adjW2)),
        "w3": bf(np32(adjW3)),
        "b1c": np.ascontiguousarray(b1.reshape(RC, P).T).astype(f32),
        "b2c": np.ascontiguousarray(np32(adjb2).reshape(R2C, P).T).astype(f32),
        "vtouW": bf(np32(vtouW)),
        "utovW": bf(np32(utovW)),
        "uupdW": bf(np32(uupdW)),
        "vupdW": bf(np32(vupdW)),
    }
    in_maps = []
    for c in range(N_CORES):
        m = dict(shared)
        m["euT_in"] = bf(eu[c * S : (c + 1) * S].T)
        in_maps.append(m)

    nc = _get_nc(b3_val)
    res = run_bass_kernel_spmd(nc, in_maps, core_ids=list(range(N_CORES)))
    eu_full = np.concatenate([res.results[c]["eu_out"] for c in range(N_CORES)], axis=0)
    adj_full = np.concatenate([res.results[c]["adj_out"] for c in range(N_CORES)], axis=0)
    ev_full = np.concatenate([res.results[c]["ev_out"] for c in range(N_CORES)], axis=0)
    return eu_full.astype(f32), ev_full.astype(f32), adj_full.astype(f32)


# revision 14
# speedup vs baseline: 1.0331x; 1.0050x over previous
"""Bass/Tile TRN2 kernel for nn_BoxPairHead (bipartite GNN message passing).

Strategy (8 NeuronCores, u-dim row-sharded, 32 u rows per core):
  - Pair-MLP layer 1 is linear in the concat -> decompose:
      pair @ W1 = (eu @ W1a)[u] + (ev @ W1b)[v]
    so the giant [256*512, 1024] x [1024, 1024] matmul collapses into two
    small matmuls (AT, BT, kept transposed: [R, *]) plus a per-u broadcast
    add + relu done on ACT/DVE with per-partition bias.
  - Layer 2 stays transposed: C.T = W2.T @ h_u  (lhsT = W2 natural chunks).
  - Layer 3: per-u columns adjT[:, u] via lhsT = gT blocks, rhs = W3 chunk
    (N=1 matmuls are nearly free); sigmoid drains [128, 4] -> adjT.
  - v->u message: msg_vu = adj @ Mv (lhsT = adjT).
  - u->v message: partial = adj.T @ Mu per core; bf16 ReduceScatter over
    the 8 cores gives each core its 64-row v-slice of the summed
    [512, 1024]; LN + v-update are computed on the slice only; iter 1
    AllGathers the updated ev (bf16) for the next iteration's BT/Mv, and
    each core keeps its own slice for the v-update lhsT (no core-dependent
    indexing needed).
  - All matmuls bf16 (PSUM accumulation fp32); LayerNorm stats fp32.
"""

import sys

sys.path.insert(0, "/opt/trn_rl_repo")

import numpy as np
import ml_dtypes

import concourse.bass as bass
from concourse import bacc, mybir
from concourse.tile import TileContext
from concourse.bass_utils import run_bass_kernel_spmd
from concourse.masks import make_identity

F32 = mybir.dt.float32
BF16 = mybir.dt.bfloat16
AF = mybir.ActivationFunctionType
OP = mybir.AluOpType

N_CORES = 8
NU, NV, K, R, NUM_ITER = 256, 512, 512, 1024, 2
R2 = R // 2  # 512
S = NU // N_CORES  # 32 u rows per core
SV = NV // N_CORES  # 64 v rows per core
P = 128
KC = K // P  # 4 contraction chunks over the encoding dim
RC = R // P  # 8 chunks over R
R2C = R2 // P  # 4 chunks over R/2
VBN = NV // P  # 4 v blocks
NB = R // 512  # 2 free-dim blocks of 512 over R
EPS = 1e-5

BF16_NP = ml_dtypes.bfloat16


class _Balancer:
    """Round-robin DVE/ACT picker weighted by estimated op cost."""

    def __init__(self, nc):
        self.nc = nc
        self.busy = {"v": 0.0, "s": 0.0}

    def pick(self, v_cost, s_cost):
        if self.busy["v"] + v_cost <= self.busy["s"] + s_cost:
            self.busy["v"] += v_cost
            return self.nc.vector
        self.busy["s"] += s_cost
        return self.nc.scalar


def _build(b3_val: float):
    nc = bacc.Bacc("TRN2", target_bir_lowering=False, debug=False, num_devices=N_CORES)

    # ---- kernel I/O ----
    euT_in = nc.dram_tensor("euT_in", [K, S], BF16, kind="ExternalInput")
    evT_in = nc.dram_tensor("evT_in", [K, NV], BF16, kind="ExternalInput")
    w1a_in = nc.dram_tensor("w1a", [K, R], BF16, kind="ExternalInput")
    w1b_in = nc.dram_tensor("w1b", [K, R], BF16, kind="ExternalInput")
    w2_in = nc.dram_tensor("w2", [R, R2], BF16, kind="ExternalInput")
    w3_in = nc.dram_tensor("w3", [R2, 1], BF16, kind="ExternalInput")
    b1_in = nc.dram_tensor("b1c", [P, RC], F32, kind="ExternalInput")
    b2_in = nc.dram_tensor("b2c", [P, R2C], F32, kind="ExternalInput")
    vtou_in = nc.dram_tensor("vtouW", [K, R], BF16, kind="ExternalInput")
    utov_in = nc.dram_tensor("utovW", [K, R], BF16, kind="ExternalInput")
    uupd_in = nc.dram_tensor("uupdW", [K + R, K], BF16, kind="ExternalInput")
    vupd_in = nc.dram_tensor("vupdW", [K + R, K], BF16, kind="ExternalInput")

    eu_out = nc.dram_tensor("eu_out", [S, K], F32, kind="ExternalOutput")
    adj_out = nc.dram_tensor("adj_out", [S, NV], F32, kind="ExternalOutput")
    ev_out = nc.dram_tensor("ev_out", [SV, K], F32, kind="ExternalOutput")

    with TileContext(nc) as tc:
        with (
            tc.tile_pool(name="singles", bufs=1) as singles,
            tc.tile_pool(name="work", bufs=2) as work,
            tc.tile_pool(name="stats", bufs=4) as stats_pool,
            tc.tile_pool(name="psum", bufs=1, space="PSUM") as psum,
            tc.tile_pool(name="dram", bufs=2, space="DRAM") as dram,
        ):
            bal = _Balancer(nc)

            # ---- inputs first (pair loop needs them), then weights ----
            def load3(name, src, c, f, dtype=BF16):
                t = singles.tile([P, c, f], dtype, name=name)
                nc.sync.dma_start(out=t, in_=src[:, :].rearrange("(c p) f -> p c f", p=P))
                return t

            euT = work.tile([P, KC, S], BF16, tag="euT")
            nc.sync.dma_start(out=euT, in_=euT_in[:, :].rearrange("(c p) s -> p c s", p=P))
            w1a = load3("w1a_sb", w1a_in, KC, R)
            evT = work.tile([P, KC, NV], BF16, tag="evT")
            nc.sync.dma_start(out=evT, in_=evT_in[:, :].rearrange("(c p) v -> p c v", p=P))
            w1b = load3("w1b_sb", w1b_in, KC, R)
            w2 = load3("w2_sb", w2_in, RC, R2)
            w3 = load3("w3_sb", w3_in, R2C, 1)
            b1 = singles.tile([P, RC], F32)
            nc.sync.dma_start(out=b1, in_=b1_in[:, :])
            b2 = singles.tile([P, R2C], F32)
            nc.sync.dma_start(out=b2, in_=b2_in[:, :])
            vtou = load3("vtou_sb", vtou_in, KC, R)
            utov = load3("utov_sb", utov_in, KC, R)
            uupd = load3("uupd_sb", uupd_in, KC + RC, K)
            vupd = load3("vupd_sb", vupd_in, KC + RC, K)

            iden = singles.tile([P, P], BF16)
            make_identity(nc, iden)
            eps_t = singles.tile([P, 1], F32)
            nc.vector.memset(eps_t, EPS)
            b3_t = singles.tile([P, 1], F32)
            nc.vector.memset(b3_t, b3_val)

            # ---- helpers ----
            def drain(dst, src, bias_col=None, relu=False, v_cost=0.6, s_cost=0.72):
                """PSUM/SBUF -> SBUF elementwise drain, optional +bias (per
                partition [P,1]) and relu, on whichever of DVE/ACT is freer."""
                eng = bal.pick(v_cost, s_cost)
                if eng is nc.vector:
                    if relu:
                        nc.vector.tensor_scalar(
                            out=dst, in0=src,
                            scalar1=bias_col if bias_col is not None else 0.0,
                            scalar2=0.0,
                            op0=OP.add, op1=OP.max,
                        )
                    elif bias_col is not None:
                        nc.vector.tensor_scalar_add(out=dst, in0=src, scalar1=bias_col)
                    else:
                        nc.vector.tensor_copy(out=dst, in_=src)
                else:
                    if relu:
                        nc.scalar.activation(
                            out=dst, in_=src, func=AF.Relu,
                            bias=bias_col if bias_col is not None else 0.0,
                        )
                    elif bias_col is not None:
                        nc.scalar.activation(out=dst, in_=src, func=AF.Identity, bias=bias_col)
                    else:
                        nc.scalar.copy(out=dst, in_=src)

            def transpose_to(dst, src):
                """src [p<=128, f<=128] SBUF bf16 -> dst [f, p] SBUF bf16."""
                pp = src.shape[0]
                ff = src.shape[-1]
                tp = psum.tile([P, P], src.dtype, tag="tp", bufs=1)
                nc.tensor.transpose(tp[:ff, :pp], src, iden[:pp, :pp])
                drain(dst, tp[:ff, :pp], v_cost=0.2, s_cost=0.45)

            def layer_norm(srcs, rows, outs):
                """LayerNorm over the concatenation of srcs (each [rows, <=512])
                along the free axis. outs: list of dst-lists, each aligned
                with srcs (multiple dst dtypes supported)."""
                nsub = len(srcs)
                st = stats_pool.tile([P, nsub, 6], F32, tag="st")
                for i, s in enumerate(srcs):
                    nc.vector.bn_stats(out=st[:rows, i, :], in_=s)
                mv = stats_pool.tile([P, 2], F32, tag="mv")
                nc.vector.bn_aggr(out=mv[:rows], in_=st[:rows])
                rstd = stats_pool.tile([P, 1], F32, tag="rstd")
                nc.scalar.activation(
                    out=rstd[:rows], in_=mv[:rows, 1:2], func=AF.Sqrt, bias=eps_t[:rows]
                )
                nc.vector.reciprocal(out=rstd[:rows], in_=rstd[:rows])
                n_apply = sum(len(d) for d in outs)
                nmr = None
                if n_apply > 1:
                    nmr = stats_pool.tile([P, 1], F32, tag="nmr")
                    nc.vector.tensor_scalar(
                        out=nmr[:rows], in0=mv[:rows, 0:1],
                        scalar1=rstd[:rows], scalar2=-1.0,
                        op0=OP.mult, op1=OP.mult,
                    )
                k = 0
                for dsts in outs:
                    for i, s in enumerate(srcs):
                        if nmr is None or k % 2 == 0:
                            nc.vector.tensor_scalar(
                                out=dsts[i], in0=s,
                                scalar1=mv[:rows, 0:1], scalar2=rstd[:rows],
                                op0=OP.subtract, op1=OP.mult,
                            )
                        else:
                            nc.scalar.activation(
                                out=dsts[i], in_=s, func=AF.Identity,
                                bias=nmr[:rows], scale=rstd[:rows],
                            )
                        k += 1

            # ---- tiny warm-up collective: absorbs cross-core startup skew
            # and ncfw dispatch warmup while the PE runs the pair loop ----
            bar_sb = singles.tile([1, 16], BF16)
            nc.vector.memset(bar_sb, 0.0)
            bar_in = dram.tile([1, 16], BF16, tag="bar_in", bufs=1)
            nc.sync.dma_start(out=bar_in[:, :], in_=bar_sb)
            bar_out = dram.tile([N_CORES, 16], BF16, tag="bar_out", bufs=1, addr_space="Shared")
            nc.gpsimd.collective_compute(
                "AllGather",
                OP.bypass,
                replica_groups=[list(range(N_CORES))],
                ins=[bar_in[:, :]],
                outs=[bar_out[:, :]],
            )

            pid = nc.sync.partition_id()
            voff = pid * SV

            # local v-slice of evT (columns [voff:voff+SV]) via dynamic DMA
            evTs = work.tile([P, KC, SV], BF16, tag="evTs")
            nc.sync.dma_start(
                out=evTs, in_=evT[:, :, bass.ds(voff, SV)]
            )

            def compute_AT():
                # AT[r, u] = (eu @ W1a).T, fp32 (used as a bias operand)
                AT = work.tile([P, RC, S], F32, tag="AT", bufs=2, name="AT")
                for rb in range(RC):
                    pa = psum.tile([P, S], F32, tag="tp", bufs=1, name="pa")
                    for kc in range(KC):
                        nc.tensor.matmul(
                            pa, w1a[:, kc, bass.ts(rb, P)], euT[:, kc, :],
                            start=(kc == 0), stop=(kc == KC - 1),
                        )
                    drain(AT[:, rb, :], pa, v_cost=0.1, s_cost=0.35)
                return AT

            AT = compute_AT()

            # ================= iterations =================
            for it in range(NUM_ITER):
                last = it == NUM_ITER - 1

                # ---- BT[r, v] = (ev @ W1b).T + b1, pre-relu ---- [P, RC, NV]
                BT = work.tile([P, RC, NV], BF16, tag="BT", bufs=1)
                for rb in range(RC):
                    pb = psum.tile([P, 512], F32, tag="big", bufs=5)
                    for kc in range(KC):
                        nc.tensor.matmul(
                            pb, w1b[:, kc, bass.ts(rb, P)], evT[:, kc, :],
                            start=(kc == 0), stop=(kc == KC - 1),
                        )
                    drain(BT[:, rb, :], pb, bias_col=b1[:, rb : rb + 1])

                # ---- Mv[v, r] = relu(ev @ vtouW) ---- [P, VBN, R]
                Mv = work.tile([P, VBN, R], BF16, tag="Mv", bufs=1)
                for vb in range(VBN):
                    for nb in range(NB):
                        pm = psum.tile([P, 512], F32, tag="big", bufs=5)
                        for kc in range(KC):
                            nc.tensor.matmul(
                                pm, evT[:, kc, bass.ts(vb, P)], vtou[:, kc, bass.ts(nb, 512)],
                                start=(kc == 0), stop=(kc == KC - 1),
                            )
                        drain(Mv[:, vb, bass.ts(nb, 512)], pm, relu=True)

                # ---- pair-MLP u loop -> adjT ----
                adj_bf = work.tile([S, NV], BF16, tag="adj_bf", bufs=1)
                adjT = work.tile([P, VBN, S], BF16, tag="adjT", bufs=1)
                for u in range(S):
                    hT = work.tile([P, RC, NV], BF16, tag="hT", bufs=3)
                    for rc in range(RC):
                        drain(
                            hT[:, rc, :], BT[:, rc, :],
                            bias_col=AT[:, rc, u : u + 1], relu=True,
                            v_cost=0.33, s_cost=0.72,
                        )
                    gT = work.tile([P, R2C, NV], BF16, tag="gT")
                    for mb in range(R2C):
                        pc = psum.tile([P, 512], F32, tag="big", bufs=5)
                        for rc in range(RC):
                            nc.tensor.matmul(
                                pc, w2[:, rc, bass.ts(mb, P)], hT[:, rc, :],
                                start=(rc == 0), stop=(rc == RC - 1),
                            )
                        drain(gT[:, mb, :], pc, bias_col=b2[:, mb : mb + 1], relu=True)
                    pcol = psum.tile([P, VBN], F32, tag="row", bufs=2)
                    for vb in range(VBN):
                        for mb in range(R2C):
                            nc.tensor.matmul(
                                pcol[:, vb : vb + 1],
                                gT[:, mb, bass.ts(vb, P)], w3[:, mb, :],
                                start=(mb == 0), stop=(mb == R2C - 1),
                            )
                    nc.scalar.activation(
                        out=adjT[:, :, u], in_=pcol, func=AF.Sigmoid, bias=b3_t
                    )
                    bal.busy["s"] += 0.3

                # ---- adj natural (+ output) from adjT ----
                for vb in range(VBN):
                    transpose_to(adj_bf[:, bass.ts(vb, P)], adjT[:, vb, :])
                if last:
                    adj_f32 = work.tile([S, NV], F32, tag="adj_f32", bufs=1)
                    nc.vector.tensor_copy(out=adj_f32, in_=adj_bf)
                    nc.sync.dma_start(out=adj_out[:, :], in_=adj_f32)

                # ---- AllGather adj rows (factor 1 of the u->v reduction);
                # overlaps the msg_vu / u-update chain ----
                cc_adj_in = dram.tile([S, NV], BF16, tag="cc_adj_in")
                nc.sync.dma_start(out=cc_adj_in[:, :], in_=adj_bf)
                cc_adj = dram.tile([NU, NV], BF16, tag="cc_adj", addr_space="Shared")
                nc.gpsimd.collective_compute(
                    "AllGather",
                    OP.bypass,
                    replica_groups=[list(range(N_CORES))],
                    ins=[cc_adj_in[:, :]],
                    outs=[cc_adj[:, :]],
                )

                # ---- msg_vu = LN(adj @ Mv) ---- rows = S
                pmv = []
                for nb in range(NB):
                    pn = psum.tile([S, 512], F32, tag="row", bufs=2)
                    for vb in range(VBN):
                        nc.tensor.matmul(
                            pn, adjT[:, vb, :], Mv[:, vb, bass.ts(nb, 512)],
                            start=(vb == 0), stop=(vb == VBN - 1),
                        )
                    pmv.append(pn)
                msgvu = work.tile([S, R], BF16, tag="msgvu", bufs=1)
                layer_norm(pmv, S, [[msgvu[:, bass.ts(nb, 512)] for nb in range(NB)]])

                # ---- u update: eu = LN([eu, msg_vu] @ uupdW) ----
                msgvuT = work.tile([P, RC, S], BF16, tag="msgvuT", bufs=1)
                for rc in range(RC):
                    transpose_to(msgvuT[:, rc, :], msgvu[:, bass.ts(rc, P)])
                pe = psum.tile([S, K], F32, tag="row", bufs=2)
                for j in range(KC + RC):
                    lhsT = euT[:, j, :] if j < KC else msgvuT[:, j - KC, :]
                    nc.tensor.matmul(
                        pe, lhsT, uupd[:, j, :],
                        start=(j == 0), stop=(j == KC + RC - 1),
                    )
                eu_nat = work.tile([S, K], BF16, tag="eu_nat", bufs=1)
                ln_outs = [[eu_nat[:, :]]]
                if last:
                    eu_f32 = work.tile([S, K], F32, tag="eu_f32", bufs=1)
                    ln_outs.append([eu_f32[:, :]])
                layer_norm([pe], S, ln_outs)
                if last:
                    nc.sync.dma_start(out=eu_out[:, :], in_=eu_f32)

                euT2 = work.tile([P, KC, S], BF16, tag="euT")
                for kc in range(KC):
                    transpose_to(euT2[:, kc, :], eu_nat[:, bass.ts(kc, P)])
                euT = euT2
                if not last:
                    AT = compute_AT()

                # ---- Mu = relu(eu_new @ utovW) ---- [S, R]
                Mu = work.tile([S, R], BF16, tag="Mu", bufs=1)
                for nb in range(NB):
                    pu = psum.tile([S, 512], F32, tag="row", bufs=2)
                    for kc in range(KC):
                        nc.tensor.matmul(
                            pu, euT[:, kc, :], utov[:, kc, bass.ts(nb, 512)],
                            start=(kc == 0), stop=(kc == KC - 1),
                        )
                    drain(Mu[:, bass.ts(nb, 512)], pu, relu=True, v_cost=0.3, s_cost=0.72)

                # ---- AllGather Mu (factor 2), then recompute only the local
                # v-slice of msg_uv = adj_full[:, voff:voff+SV].T @ Mu_full ----
                cc_mu_in = dram.tile([S, R], BF16, tag="cc_mu_in")
                nc.sync.dma_start(out=cc_mu_in[:, :], in_=Mu)
                cc_mu = dram.tile([NU, R], BF16, tag="cc_mu", addr_space="Shared")
                nc.gpsimd.collective_compute(
                    "AllGather",
                    OP.bypass,
                    replica_groups=[list(range(N_CORES))],
                    ins=[cc_mu_in[:, :]],
                    outs=[cc_mu[:, :]],
                )
                muF = work.tile([P, NU // P, R], BF16, tag="muF", bufs=1)
                nc.sync.dma_start(
                    out=muF, in_=cc_mu[:, :].rearrange("(c p) r -> p c r", p=P)
                )
                adjS = work.tile([P, NU // P, SV], BF16, tag="adjS", bufs=1)
                nc.sync.dma_start(
                    out=adjS,
                    in_=cc_adj[:, :].rearrange("(c p) v -> p c v", p=P)[
                        :, :, bass.ds(voff, SV)
                    ],
                )
                pmuv = []
                for nb in range(NB):
                    pw = psum.tile([SV, 512], F32, tag="row", bufs=2)
                    for c in range(NU // P):
                        nc.tensor.matmul(
                            pw, adjS[:, c, :], muF[:, c, bass.ts(nb, 512)],
                            start=(c == 0), stop=(c == NU // P - 1),
                        )
                    pmuv.append(pw)

                # ---- LN of the local v-slice of msg_uv ----
                mslice_n = work.tile([SV, R], BF16, tag="mslice_n", bufs=1)
                layer_norm(
                    pmuv,
                    SV,
                    [[mslice_n[:, bass.ts(nb, 512)] for nb in range(NB)]],
                )
                msguvT = work.tile([P, RC, SV], BF16, tag="msguvT", bufs=1)
                for rc in range(RC):
                    transpose_to(msguvT[:, rc, :], mslice_n[:, bass.ts(rc, P)])

                # ---- v update (local slice): ev_s = LN([ev_s, msg_uv_s] @ vupdW) ----
                pv = psum.tile([SV, K], F32, tag="row", bufs=2)
                for j in range(KC + RC):
                    lhsT = evTs[:, j, :] if j < KC else msguvT[:, j - KC, :]
                    nc.tensor.matmul(
                        pv, lhsT, vupd[:, j, :],
                        start=(j == 0), stop=(j == KC + RC - 1),
                    )
                if last:
                    evs_f32 = work.tile([SV, K], F32, tag="evs_f32", bufs=1)
                    layer_norm([pv], SV, [[evs_f32[:, :]]])
                    nc.sync.dma_start(out=ev_out[:, :], in_=evs_f32)
                else:
                    evs_nat = work.tile([SV, K], BF16, tag="evs_nat", bufs=1)
                    layer_norm([pv], SV, [[evs_nat[:, :]]])
                    # AllGather the updated ev for the next iteration's BT/Mv
                    cc_agin = dram.tile([SV, K], BF16, tag="cc_agin")
                    nc.sync.dma_start(out=cc_agin[:, :], in_=evs_nat)
                    cc_ag = dram.tile([NV, K], BF16, tag="cc_ag", addr_space="Shared")
                    nc.gpsimd.collective_compute(
                        "AllGather",
                        OP.bypass,
                        replica_groups=[list(range(N_CORES))],
                        ins=[cc_agin[:, :]],
                        outs=[cc_ag[:, :]],
                    )
                    ev_nat = work.tile([P, VBN, K], BF16, tag="ev_nat", bufs=1)
                    nc.sync.dma_start(
                        out=ev_nat, in_=cc_ag[:, :].rearrange("(vb p) k -> p vb k", p=P)
                    )
                    evT2 = work.tile([P, KC, NV], BF16, tag="evT")
                    for kc in range(KC):
                        for vb in range(VBN):
                            transpose_to(
                                evT2[:, kc, bass.ts(vb, P)],
                                ev_nat[:, vb, bass.ts(kc, P)],
                            )
                    evT = evT2
                    # next iteration's local evT slice (lhsT of the v update)
                    evTs2 = work.tile([P, KC, SV], BF16, tag="evTs")
                    nc.sync.dma_start(
                        out=evTs2, in_=evT[:, :, bass.ds(voff, SV)]
                    )
                    evTs = evTs2

    nc.compile()
    return nc


_CACHE = {}


def _get_nc(b3_val: float):
    key = float(b3_val)
    if key not in _CACHE:
        _CACHE[key] = _build(key)
    return _CACHE[key]


def kernel(
    encodings_u, encodings_v, adjW1, adjb1, adjW2, adjb2, adjW3, adjb3,
    utovW, utovb, vtouW, vtoub, utov_g, utov_b, vtou_g, vtou_b,
    uupdW, uupd_g, uupd_b, vupdW, vupd_g, vupd_b,
):
    f32 = np.float32

    def np32(x):
        return np.asarray(x, dtype=f32)

    # The device kernel folds the trivial (zero/one) affine params away;
    # assert they really are trivial for this problem instance.
    for name, arr, val in [
        ("utovb", utovb, 0.0), ("vtoub", vtoub, 0.0),
        ("utov_g", utov_g, 1.0), ("utov_b", utov_b, 0.0),
        ("vtou_g", vtou_g, 1.0), ("vtou_b", vtou_b, 0.0),
        ("uupd_g", uupd_g, 1.0), ("uupd_b", uupd_b, 0.0),
        ("vupd_g", vupd_g, 1.0), ("vupd_b", vupd_b, 0.0),
    ]:
        if not np.allclose(np32(arr), val, atol=1e-30):
            raise NotImplementedError(f"nontrivial {name} not supported")

    eu = np32(encodings_u)
    ev = np32(encodings_v)
    W1 = np32(adjW1)
    b1 = np32(adjb1)
    b3_val = float(np32(adjb3).reshape(-1)[0])

    def bf(x):
        return np.ascontiguousarray(x).astype(BF16_NP)

    evT = bf(ev.T)
    shared = {
        "evT_in": evT,
        "w1a": bf(W1[:K]),
        "w1b": bf(W1[K:]),
        "w2": bf(np32(adjW2)),
        "w3": bf(np32(adjW3)),
        "b1c": np.ascontiguousarray(b1.reshape(RC, P).T).astype(f32),
        "b2c": np.ascontiguousarray(np32(adjb2).reshape(R2C, P).T).astype(f32),
        "vtouW": bf(np32(vtouW)),
        "utovW": bf(np32(utovW)),
        "uupdW": bf(np32(uupdW)),
        "vupdW": bf(np32(vupdW)),
    }
    in_maps = []
    for c in range(N_CORES):
        m = dict(shared)
        m["euT_in"] = bf(eu[c * S : (c + 1) * S].T)
        in_maps.append(m)

    nc = _get_nc(b3_val)
    res = run_bass_kernel_spmd(nc, in_maps, core_ids=list(range(N_CORES)))
    eu_full = np.concatenate([res.results[c]["eu_out"] for c in range(N_CORES)], axis=0)
    adj_full = np.concatenate([res.results[c]["adj_out"] for c in range(N_CORES)], axis=0)
    ev_full = np.concatenate([res.results[c]["ev_out"] for c in range(N_CORES)], axis=0)
    return eu_full.astype(f32), ev_full.astype(f32), adj_full.astype(f32)
